# revision 34
# baseline (speedup 1.0000x reference)
"""Multi-head attention (B=4, S=2048, D=768, H=12) on 8 TRN2 NeuronCores.

Sharding: 48 (batch, head) units -> core c handles batch c//2, heads
6*(c%2) .. 6*(c%2)+5 (tensor-parallel over heads). Each core computes a
partial output projection; the host sums the two partials per batch and
adds the bias.

v2 over the baseline:
- Attention processes HEAD PAIRS (even/odd head of each 128-dim m tile).
  The two K=64 logit matmuls occupy disjoint PE row groups (rows 0-63 /
  64-127) so the hardware runs them concurrently (row tiling).
- exp is SPLIT across two engines: head A's exp runs on the Scalar
  engine (table exp), head B's exp mostly runs on the Vector engine via
  the fp16-bitcast trick: i16 = (int16)(1024*(logit*scale*log2e + 15) + C),
  bitcast to fp16 ~= 2^t (max rel err ~3%, softmax-ratio cancels bias).
  One fused DVE tensor_scalar per tile -> DVE becomes a second exp pipe.
- reciprocal -> reciprocal_approx_fast (single DVE op, ~5x faster).
- Q/K projections for m=1,2 and the first 8 output projections are
  interleaved INTO attention blocks as [128,512] PSUM chunks that slot
  into the gaps the exp pipeline leaves on the PE.
"""

import numpy as np

import concourse.bacc as bacc
import concourse.mybir as mybir
from concourse import tile
from concourse.bass_utils import run_bass_kernel_spmd

B, S, D, H = 4, 2048, 768, 12
DEPTH = D // H  # 64
HPC = H // 2  # heads per core: 6
HD = HPC * DEPTH  # per-core projected dim: 384
EC = D // 128  # e chunks: 6
MT = HD // 128  # d tiles: 3
ST = S // 128  # s tiles: 16
QH = 2  # q halves in attention
QHS = S // QH  # 1024

f32 = mybir.dt.float32
fp16 = mybir.dt.float16
i16 = mybir.dt.int16
AF = mybir.ActivationFunctionType
ALU = mybir.AluOpType

SCALE = 1.0 / np.sqrt(DEPTH)
# DVE fast-exp constants: exp(x) ~ bitcast_fp16(int16(1024*(x*SCALE*log2e + 15) - 44))
EXP_A = float(SCALE * np.log2(np.e) * 1024.0)
EXP_B = float(15.0 * 1024.0 - 44.0)
_CACHE = {}


def _build():
    if "nc" in _CACHE:
        return _CACHE["nc"]
    nc = bacc.Bacc("TRN2", target_bir_lowering=False, debug=False, num_devices=8)
    qt = nc.dram_tensor("qt", [D, S], fp16, kind="ExternalInput").ap()
    kt = nc.dram_tensor("kt", [D, S], fp16, kind="ExternalInput").ap()
    vt = nc.dram_tensor("vt", [D, S], fp16, kind="ExternalInput").ap()
    wqt = nc.dram_tensor("wqt", [D, HD], fp16, kind="ExternalInput").ap()
    wkt = nc.dram_tensor("wkt", [D, HD], fp16, kind="ExternalInput").ap()
    wvt = nc.dram_tensor("wvt", [D, HD], fp16, kind="ExternalInput").ap()
    wot = nc.dram_tensor("wot", [HD, D], fp16, kind="ExternalInput").ap()
    y = nc.dram_tensor("y", [S, D], f32, kind="ExternalOutput").ap()

    with tile.TileContext(nc) as tc:
        with (
            tc.tile_pool(name="wp", bufs=3) as wp,
            tc.tile_pool(name="wop", bufs=1) as wop,
            tc.tile_pool(name="xp", bufs=3 * EC) as xp,
            tc.tile_pool(name="qk", bufs=2 * MT) as qkp,
            tc.tile_pool(name="vg", bufs=ST) as vgp,
            tc.tile_pool(name="ot", bufs=MT) as otp,
            tc.tile_pool(name="ep", bufs=8) as epp,
            tc.tile_pool(name="sm", bufs=2) as smp,
            tc.tile_pool(name="yp", bufs=2) as ypp,
        ):
            # ---- persistent SBUF tensors ----
            qht = [qkp.tile([128, S], fp16, tag="qk", name=f"qht{i}") for i in range(MT)]
            kht = [qkp.tile([128, S], fp16, tag="qk", name=f"kht{i}") for i in range(MT)]
            vaug = [vgp.tile([128, HPC, DEPTH + 1], fp16, tag="vg", name=f"vaug{i}") for i in range(ST)]
            outt = [otp.tile([128, S], fp16, tag="ot", name=f"outt{i}") for i in range(MT)]

            wot_sb = wop.tile([128, MT, D], fp16, tag="wot")
            zt = wop.tile([128, DEPTH + 1], fp16, tag="zt")
            nc.vector.memset(zt[:], 0.0)

            def load_w(wdram, nm):
                w_sb = wp.tile([128, EC, HD], fp16, tag="w", name=f"w_{nm}")
                for ci in range(EC):
                    nc.sync.dma_start(
                        out=w_sb[:, ci, :],
                        in_=wdram[ci * 128 : (ci + 1) * 128, :],
                    )
                return w_sb

            def load_x(xdram, nm):
                xc = [
                    xp.tile([128, S], fp16, tag="x", name=f"x{nm}_{i}")
                    for i in range(EC)
                ]
                for ci in range(EC):
                    nc.sync.dma_start(
                        out=xc[ci][:], in_=xdram[ci * 128 : (ci + 1) * 128, :]
                    )
                return xc

            with (
                tc.tile_pool(name="plog", bufs=2, space="PSUM") as plog,
                tc.tile_pool(name="pacc", bufs=2, space="PSUM") as pacc,
            ):

                def proj_qk_m(name, w_sb, xc, dst, m):
                    # full Q/K d-tile (m): both 1024-halves (startup only)
                    for sh in range(2):
                        proj_qk_chunk(name, w_sb, xc, dst, m, sh, 0)
                        proj_qk_chunk(name, w_sb, xc, dst, m, sh, 1)

                def proj_qk_chunk(name, w_sb, xc, dst, m, sh, n):
                    # one [128, 512] chunk of a Q/K projection d-tile
                    off = sh * 1024 + n * 512
                    pt = plog.tile(
                        [128, 512], f32, tag="plog", name=f"p{name}{m}_{sh}_{n}"
                    )
                    for ci in range(EC):
                        nc.tensor.matmul(
                            pt[:],
                            w_sb[:, ci, m * 128 : (m + 1) * 128],
                            xc[ci][:, off : off + 512],
                            start=(ci == 0),
                            stop=(ci == EC - 1),
                        )
                    with nc.allow_low_precision(reason="fp16 pipeline"):
                        nc.scalar.copy(dst[m][:, off : off + 512], pt[:])

                def proj_v_tile(wv_sb, xc, s):
                    pv = plog.tile([128, HD], f32, tag="plog", name=f"pv{s}")
                    for ci in range(EC):
                        nc.tensor.matmul(
                            pv[:],
                            xc[ci][:, s * 128 : (s + 1) * 128],
                            wv_sb[:, ci, :],
                            start=(ci == 0),
                            stop=(ci == EC - 1),
                        )
                    with nc.allow_low_precision(reason="fp16 pipeline"):
                        nc.vector.tensor_copy(
                            vaug[s][:, :, 0:DEPTH],
                            pv[:].rearrange("p (h d) -> p h d", d=DEPTH),
                        )
                    nc.vector.memset(vaug[s][:, :, DEPTH : DEPTH + 1], 1.0)

                def outproj(s, pool, on_act=False):
                    py = pool.tile([128, D], f32, tag=pool.name, name=f"py{s}")
                    for m in range(MT):
                        for n0, n1 in ((0, 512), (512, 768)):
                            nc.tensor.matmul(
                                py[:, n0:n1],
                                outt[m][:, s * 128 : (s + 1) * 128],
                                wot_sb[:, m, n0:n1],
                                start=(m == 0),
                                stop=(m == MT - 1),
                            )
                    ty = ypp.tile([128, D], f32, tag="y", name=f"ty{s}")
                    if on_act:
                        nc.scalar.copy(ty[:], py[:])
                    else:
                        nc.vector.tensor_copy(ty[:], py[:])
                    nc.sync.dma_start(out=y[s * 128 : (s + 1) * 128, :], in_=ty[:])

                def attnpair(m, qh, jit_v=None, extra=None, fill=0, tail_norm=False):
                    # heads A=2m (rows 0:64), B=2m+1 (rows 64:128)
                    hA, hB = 2 * m, 2 * m + 1
                    q0 = qh * QHS
                    accA = pacc.tile(
                        [DEPTH + 1, QHS], f32, tag="pacc", name=f"accA{m}_{qh}"
                    )
                    accB = pacc.tile(
                        [DEPTH + 1, QHS], f32, tag="pacc", name=f"accB{m}_{qh}"
                    )

                    def lg(kt_i):
                        lpA = plog.tile(
                            [128, QHS], f32, tag="plog", name=f"lpA{m}_{qh}_{kt_i}"
                        )
                        lpB = plog.tile(
                            [128, QHS], f32, tag="plog", name=f"lpB{m}_{qh}_{kt_i}"
                        )
                        for n in range(2):
                            for base, lp in ((0, lpA), (64, lpB)):
                                nc.tensor.matmul(
                                    lp[:, n * 512 : (n + 1) * 512],
                                    kht[m][
                                        base : base + 64,
                                        kt_i * 128 : (kt_i + 1) * 128,
                                    ],
                                    qht[m][
                                        base : base + 64,
                                        q0 + n * 512 : q0 + (n + 1) * 512,
                                    ],
                                    start=True,
                                    stop=True,
                                )
                        etA = epp.tile(
                            [128, QHS], fp16, tag="ep", name=f"etA{m}_{qh}_{kt_i}"
                        )
                        etB = epp.tile(
                            [128, QHS], fp16, tag="ep", name=f"etB{m}_{qh}_{kt_i}"
                        )
                        with nc.allow_low_precision(reason="fp16 pipeline"):
                            nc.scalar.activation(etA[:], lpA[:], AF.Exp, scale=SCALE)
                        # head B's exp always on the DVE (2^t bitcast trick):
                        # a second exp pipe in parallel with ScalarE.
                        nc.vector.tensor_scalar(
                            out=etB[:].bitcast(i16),
                            in0=lpB[:],
                            scalar1=EXP_A,
                            scalar2=EXP_B,
                            op0=ALU.mult,
                            op1=ALU.add,
                        )
                        # Joiner: a 1-element op reading BOTH lp tiles makes
                        # their pool slots release on the same event, so the
                        # next kt's row-group matmul pair becomes ready
                        # simultaneously and issues back-to-back (concurrent
                        # row tiling needs adjacent issue).
                        jd = smp.tile([1, 2], f32, tag="j", name=f"j{m}_{qh}_{kt_i}")
                        nc.vector.tensor_copy(jd[:, 0:1], lpA[0:1, 0:1])
                        nc.vector.tensor_copy(jd[:, 1:2], lpB[0:1, 0:1])
                        return etA, etB

                    def av(kt_i, ets):
                        etA, etB = ets
                        for acc, h, et in ((accA, hA, etA), (accB, hB, etB)):
                            for n in range(2):
                                nc.tensor.matmul(
                                    acc[:, n * 512 : (n + 1) * 512],
                                    vaug[kt_i][:, h, :],
                                    et[:, n * 512 : (n + 1) * 512],
                                    start=(kt_i == 0),
                                    stop=(kt_i == ST - 1),
                                )

                    def warmfill(ets):
                        # Zero-matmuls accumulating 0*et into acc: exact
                        # numeric no-ops that keep the PE array active in the
                        # exp-latency slivers, so the HAM activity monitor
                        # never re-throttles the PE clock to 1.2 GHz.
                        for i in range(fill):
                            nc.tensor.matmul(
                                accA[:, i * 512 : (i + 1) * 512],
                                zt[:],
                                ets[0][:, i * 512 : (i + 1) * 512],
                                start=False,
                                stop=False,
                                skip_group_check=True,
                            )

                    if jit_v is not None:
                        jit_v(0)
                    prev = lg(0)
                    for kt_i in range(1, ST):
                        if jit_v is not None:
                            jit_v(kt_i)
                        if extra is not None and kt_i in extra:
                            for f in extra[kt_i]:
                                f()
                        cur = lg(kt_i)
                        av(kt_i - 1, prev)
                        if fill and kt_i < ST - 1:
                            warmfill(prev)
                        prev = cur
                    av(ST - 1, prev)
                    # Stage acc out of PSUM fast (frees the banks for the
                    # next block so the PE never idles past the HAM window);
                    # the actual normalization happens in finish_norm(),
                    # emitted early in the NEXT block.
                    # Stage both accumulators out of PSUM in PARALLEL (head A
                    # via ScalarE, head B via DVE) so the acc slots free
                    # ~1.2us after the last exp and the next block's pipeline
                    # refills without a PE bubble (PE bubbles > ~1.5us here
                    # trip the HAM re-throttle and halve the PE clock).
                    soA = smp.tile([DEPTH + 1, QHS], f32, tag="so", name=f"soA{m}_{qh}")
                    soB = smp.tile([DEPTH + 1, QHS], f32, tag="so", name=f"soB{m}_{qh}")
                    nc.scalar.copy(soA[:], accA[:])
                    nc.vector.tensor_copy(soB[:], accB[:])

                    # Deferred normalization, split into small steps the
                    # caller schedules across the NEXT block's kt loop.
                    # reciprocal_approx_fast is broken on HW for partition-64
                    # sources, so the denominator row is first copied down to
                    # partition 0 with a stock DVE copy.
                    def steps(so, base, m=m, q0=q0):
                        sd = smp.tile([1, QHS], f32, tag="sd", name=f"sd{m}_{q0}_{base}")
                        r = smp.tile([1, QHS], f32, tag="r", name=f"r{m}_{q0}_{base}")
                        rb = smp.tile([64, QHS], f32, tag="rb", name=f"rb{m}_{q0}_{base}")

                        def s1():
                            nc.vector.tensor_copy(sd[:], so[DEPTH : DEPTH + 1, :])

                        def s2():
                            nc.vector.reciprocal_approx_fast(out=r[:], in_=sd[:])

                        def s3():
                            nc.gpsimd.partition_broadcast(rb[:], r[:])
                            mul_eng = nc.vector if tail_norm else nc.gpsimd
                            with nc.allow_low_precision(reason="fp16 pipeline"):
                                mul_eng.tensor_mul(
                                    outt[m][base : base + 64, q0 : q0 + QHS],
                                    so[0:DEPTH, :],
                                    rb[:],
                                )

                        return [s1, s2, s3]

                    return steps(soA, 0) + steps(soB, 64)

                # ---- emission ----
                wq_sb = load_w(wqt, "q")
                xq = load_x(qt, "q")
                wk_sb = load_w(wkt, "k")
                xk = load_x(kt, "k")
                wv_sb = load_w(wvt, "v")
                xv = load_x(vt, "v")
                nc.sync.dma_start(
                    out=wot_sb[:], in_=wot.rearrange("(m p) o -> p m o", p=128)
                )
                def wire(steps, *hooks, norm_kt=(3, 5, 7, 9, 11, 15)):
                    # steps: 6 deferred-normalize callables from the previous
                    # block; hooks: (kt, fn) pairs for this block.
                    ex = {}
                    if steps is not None:
                        for k, f in zip(norm_kt, steps):
                            ex.setdefault(k, []).append(f)
                    for k, f in hooks:
                        ex.setdefault(k, []).append(f)
                    return ex

                def projhooks(wq_s, xq_c, wk_s, xk_c, mi):
                    hooks = []
                    plan = [
                        (2, "q", 0, 0), (4, "q", 0, 1), (6, "q", 1, 0), (8, "q", 1, 1),
                        (10, "k", 0, 0), (12, "k", 0, 1), (13, "k", 1, 0), (14, "k", 1, 1),
                    ]
                    for kt, nm, sh, n in plan:
                        if nm == "q":
                            w_sb, xc, dst = wq_s, xq_c, qht
                        else:
                            w_sb, xc, dst = wk_s, xk_c, kht
                        hooks.append(
                            (kt, (lambda nm=nm, w_sb=w_sb, xc=xc, dst=dst, sh=sh, n=n:
                                  proj_qk_chunk(nm, w_sb, xc, dst, mi, sh, n)))
                        )
                    return hooks

                # Pre-warm the PE: a burst of zero-matmuls as soon as the
                # first weights land flips the HAM to 8/8 before the real
                # projections start, so they run at 2.4 GHz.
                pw = plog.tile([128, 384], f32, tag="plog", name="pw")
                for i in range(20):
                    nc.tensor.matmul(
                        pw[0:DEPTH + 1, :],
                        zt[:],
                        wq_sb[:, 0, :],
                        start=True,
                        stop=True,
                    )
                proj_qk_m("q", wq_sb, xq, qht, 0)
                proj_qk_m("k", wk_sb, xk, kht, 0)
                st = attnpair(0, 0, jit_v=lambda s: proj_v_tile(wv_sb, xv, s), fill=1)
                st = attnpair(0, 1, extra=wire(st, *projhooks(wq_sb, xq, wk_sb, xk, 1)))
                st = attnpair(1, 0, extra=wire(st, *projhooks(wq_sb, xq, wk_sb, xk, 2)))
                st = attnpair(1, 1, extra=wire(st), fill=2)
                st = attnpair(2, 0, extra=wire(st), fill=2)
                # outproj s=0..5 interleaved late into the last block (they
                # need block (2,0)'s normalize, which runs at kts 1..6 here
                # and completes by ~kt 9); s=6,7 right after the block while
                # its own normalize chain runs on DVE/GpSimd; s>=8 after.
                po = [
                    (10 + s, (lambda s=s: outproj(s, plog, on_act=True)))
                    for s in range(6)
                ]
                st = attnpair(
                    2, 1,
                    extra=wire(st, *po, norm_kt=(1, 2, 3, 4, 5, 6)),
                    fill=2,
                    tail_norm=True,
                )
                outproj(6, plog, on_act=True)
                outproj(7, plog, on_act=True)
                for f in st:
                    f()
                for s in range(8, ST):
                    outproj(s, plog, on_act=(s % 2 == 0))

    nc.compile()
    _CACHE["nc"] = nc
    return nc


def make_in_maps(v, k, q, wq, wk, wv, wo):
    f16 = lambda x: np.ascontiguousarray(x, dtype=np.float32).astype(np.float16)
    in_maps = []
    for c in range(8):
        b = c // 2
        hs = (c % 2) * HD
        in_maps.append(
            {
                "qt": f16(q[b].T),
                "kt": f16(k[b].T),
                "vt": f16(v[b].T),
                "wqt": f16(wq[hs : hs + HD, :].T),
                "wkt": f16(wk[hs : hs + HD, :].T),
                "wvt": f16(wv[hs : hs + HD, :].T),
                "wot": f16(wo[:, hs : hs + HD].T),
            }
        )
    return in_maps


def assemble(results, bo):
    y = np.empty((B, S, D), dtype=np.float32)
    for b in range(B):
        y[b] = results[2 * b]["y"] + results[2 * b + 1]["y"] + bo[None, :]
    return y


def kernel(v, k, q, wq, wk, wv, wo, bo):
    nc = _build()
    in_maps = make_in_maps(v, k, q, wq, wk, wv, wo)
    res = run_bass_kernel_spmd(nc, in_maps, list(range(8)))
    return assemble(res.results, np.asarray(bo, dtype=np.float32))


# revision 36
# speedup vs baseline: 1.1619x; 1.1619x over previous
"""Multi-head attention (B=4, S=2048, D=768, H=12) on 8 TRN2 NeuronCores.

Sharding: 48 (batch, head) units -> core c handles batch c//2, heads
6*(c%2) .. 6*(c%2)+5 (tensor-parallel over heads). Each core computes a
partial output projection; the host sums the two partials per batch and
adds the bias.

v2 over the baseline:
- Attention processes HEAD PAIRS (even/odd head of each 128-dim m tile).
  The two K=64 logit matmuls occupy disjoint PE row groups (rows 0-63 /
  64-127) so the hardware runs them concurrently (row tiling).
- exp is SPLIT across two engines: head A's exp runs on the Scalar
  engine (table exp), head B's exp mostly runs on the Vector engine via
  the fp16-bitcast trick: i16 = (int16)(1024*(logit*scale*log2e + 15) + C),
  bitcast to fp16 ~= 2^t (max rel err ~3%, softmax-ratio cancels bias).
  One fused DVE tensor_scalar per tile -> DVE becomes a second exp pipe.
- reciprocal -> reciprocal_approx_fast (single DVE op, ~5x faster).
- Q/K projections for m=1,2 and the first 8 output projections are
  interleaved INTO attention blocks as [128,512] PSUM chunks that slot
  into the gaps the exp pipeline leaves on the PE.
"""

import numpy as np

import concourse.bacc as bacc
import concourse.mybir as mybir
from concourse import tile
from concourse.bass_utils import run_bass_kernel_spmd

B, S, D, H = 4, 2048, 768, 12
DEPTH = D // H  # 64
HPC = H // 2  # heads per core: 6
HD = HPC * DEPTH  # per-core projected dim: 384
EC = D // 128  # e chunks: 6
MT = HD // 128  # d tiles: 3
ST = S // 128  # s tiles: 16
QH = 2  # q halves in attention
QHS = S // QH  # 1024

f32 = mybir.dt.float32
fp16 = mybir.dt.float16
i16 = mybir.dt.int16
AF = mybir.ActivationFunctionType
ALU = mybir.AluOpType

SCALE = 1.0 / np.sqrt(DEPTH)
# DVE fast-exp constants: exp(x) ~ bitcast_fp16(int16(1024*(x*SCALE*log2e + 15) - 44))
EXP_A = float(SCALE * np.log2(np.e) * 1024.0)
EXP_B = float(15.0 * 1024.0 - 44.0)
_CACHE = {}


def _build():
    if "nc" in _CACHE:
        return _CACHE["nc"]
    nc = bacc.Bacc("TRN2", target_bir_lowering=False, debug=False, num_devices=8)
    qt = nc.dram_tensor("qt", [D, S], fp16, kind="ExternalInput").ap()
    kt = nc.dram_tensor("kt", [D, S], fp16, kind="ExternalInput").ap()
    vt = nc.dram_tensor("vt", [D, S], fp16, kind="ExternalInput").ap()
    wqt = nc.dram_tensor("wqt", [D, HD], fp16, kind="ExternalInput").ap()
    wkt = nc.dram_tensor("wkt", [D, HD], fp16, kind="ExternalInput").ap()
    wvt = nc.dram_tensor("wvt", [D, HD], fp16, kind="ExternalInput").ap()
    wot = nc.dram_tensor("wot", [HD, D], fp16, kind="ExternalInput").ap()
    y = nc.dram_tensor("y", [S, D], f32, kind="ExternalOutput").ap()

    with tile.TileContext(nc) as tc:
        with (
            tc.tile_pool(name="wp", bufs=3) as wp,
            tc.tile_pool(name="wop", bufs=1) as wop,
            tc.tile_pool(name="xp", bufs=3 * EC) as xp,
            tc.tile_pool(name="qk", bufs=2 * MT) as qkp,
            tc.tile_pool(name="vg", bufs=ST) as vgp,
            tc.tile_pool(name="ot", bufs=MT) as otp,
            tc.tile_pool(name="ep", bufs=8) as epp,
            tc.tile_pool(name="sm", bufs=2) as smp,
            tc.tile_pool(name="yp", bufs=2) as ypp,
        ):
            # ---- persistent SBUF tensors ----
            qht = [qkp.tile([128, S], fp16, tag="qk", name=f"qht{i}") for i in range(MT)]
            kht = [qkp.tile([128, S], fp16, tag="qk", name=f"kht{i}") for i in range(MT)]
            vaug = [vgp.tile([128, HPC, DEPTH + 1], fp16, tag="vg", name=f"vaug{i}") for i in range(ST)]
            outt = [otp.tile([128, S], fp16, tag="ot", name=f"outt{i}") for i in range(MT)]

            wot_sb = wop.tile([128, MT, D], fp16, tag="wot")
            zt = wop.tile([128, DEPTH + 1], fp16, tag="zt")
            nc.vector.memset(zt[:], 0.0)

            def load_w(wdram, nm):
                w_sb = wp.tile([128, EC, HD], fp16, tag="w", name=f"w_{nm}")
                for ci in range(EC):
                    nc.sync.dma_start(
                        out=w_sb[:, ci, :],
                        in_=wdram[ci * 128 : (ci + 1) * 128, :],
                    )
                return w_sb

            def load_x(xdram, nm):
                xc = [
                    xp.tile([128, S], fp16, tag="x", name=f"x{nm}_{i}")
                    for i in range(EC)
                ]
                for ci in range(EC):
                    nc.sync.dma_start(
                        out=xc[ci][:], in_=xdram[ci * 128 : (ci + 1) * 128, :]
                    )
                return xc

            with (
                tc.tile_pool(name="plog", bufs=2, space="PSUM") as plog,
                tc.tile_pool(name="pacc", bufs=2, space="PSUM") as pacc,
            ):

                def proj_qk_m(name, w_sb, xc, dst, m):
                    # full Q/K d-tile (m): both 1024-halves (startup only)
                    for sh in range(2):
                        proj_qk_chunk(name, w_sb, xc, dst, m, sh, 0)
                        proj_qk_chunk(name, w_sb, xc, dst, m, sh, 1)

                def proj_qk_chunk(name, w_sb, xc, dst, m, sh, n):
                    # one [128, 512] chunk of a Q/K projection d-tile
                    off = sh * 1024 + n * 512
                    pt = plog.tile(
                        [128, 512], f32, tag="plog", name=f"p{name}{m}_{sh}_{n}"
                    )
                    for ci in range(EC):
                        nc.tensor.matmul(
                            pt[:],
                            w_sb[:, ci, m * 128 : (m + 1) * 128],
                            xc[ci][:, off : off + 512],
                            start=(ci == 0),
                            stop=(ci == EC - 1),
                        )
                    with nc.allow_low_precision(reason="fp16 pipeline"):
                        nc.scalar.copy(dst[m][:, off : off + 512], pt[:])

                def proj_v_tile(wv_sb, xc, s):
                    pv = plog.tile([128, HD], f32, tag="plog", name=f"pv{s}")
                    for ci in range(EC):
                        nc.tensor.matmul(
                            pv[:],
                            xc[ci][:, s * 128 : (s + 1) * 128],
                            wv_sb[:, ci, :],
                            start=(ci == 0),
                            stop=(ci == EC - 1),
                        )
                    with nc.allow_low_precision(reason="fp16 pipeline"):
                        nc.vector.tensor_copy(
                            vaug[s][:, :, 0:DEPTH],
                            pv[:].rearrange("p (h d) -> p h d", d=DEPTH),
                        )
                    nc.vector.memset(vaug[s][:, :, DEPTH : DEPTH + 1], 1.0)

                def outproj(s, pool, on_act=False):
                    py = pool.tile([128, D], f32, tag=pool.name, name=f"py{s}")
                    for m in range(MT):
                        for n0, n1 in ((0, 512), (512, 768)):
                            nc.tensor.matmul(
                                py[:, n0:n1],
                                outt[m][:, s * 128 : (s + 1) * 128],
                                wot_sb[:, m, n0:n1],
                                start=(m == 0),
                                stop=(m == MT - 1),
                            )
                    ty = ypp.tile([128, D], f32, tag="y", name=f"ty{s}")
                    if on_act:
                        nc.scalar.copy(ty[:], py[:])
                    else:
                        nc.vector.tensor_copy(ty[:], py[:])
                    nc.sync.dma_start(out=y[s * 128 : (s + 1) * 128, :], in_=ty[:])

                def attnpair(m, qh, jit_v=None, extra=None, fill=0, tail_norm=False):
                    # heads A=2m (rows 0:64), B=2m+1 (rows 64:128)
                    hA, hB = 2 * m, 2 * m + 1
                    q0 = qh * QHS
                    accA = pacc.tile(
                        [DEPTH + 1, QHS], f32, tag="pacc", name=f"accA{m}_{qh}"
                    )
                    accB = pacc.tile(
                        [DEPTH + 1, QHS], f32, tag="pacc", name=f"accB{m}_{qh}"
                    )

                    def lg(kt_i):
                        lpA = plog.tile(
                            [128, QHS], f32, tag="plog", name=f"lpA{m}_{qh}_{kt_i}"
                        )
                        lpB = plog.tile(
                            [128, QHS], f32, tag="plog", name=f"lpB{m}_{qh}_{kt_i}"
                        )
                        for n in range(2):
                            for base, lp in ((0, lpA), (64, lpB)):
                                nc.tensor.matmul(
                                    lp[:, n * 512 : (n + 1) * 512],
                                    kht[m][
                                        base : base + 64,
                                        kt_i * 128 : (kt_i + 1) * 128,
                                    ],
                                    qht[m][
                                        base : base + 64,
                                        q0 + n * 512 : q0 + (n + 1) * 512,
                                    ],
                                    start=True,
                                    stop=True,
                                )
                        etA = epp.tile(
                            [128, QHS], fp16, tag="ep", name=f"etA{m}_{qh}_{kt_i}"
                        )
                        etB = epp.tile(
                            [128, QHS], fp16, tag="ep", name=f"etB{m}_{qh}_{kt_i}"
                        )
                        with nc.allow_low_precision(reason="fp16 pipeline"):
                            nc.scalar.activation(etA[:], lpA[:], AF.Exp, scale=SCALE)
                        # head B's exp always on the DVE (2^t bitcast trick):
                        # a second exp pipe in parallel with ScalarE.
                        nc.vector.tensor_scalar(
                            out=etB[:].bitcast(i16),
                            in0=lpB[:],
                            scalar1=EXP_A,
                            scalar2=EXP_B,
                            op0=ALU.mult,
                            op1=ALU.add,
                        )
                        return etA, etB

                    def av(kt_i, ets):
                        etA, etB = ets
                        for acc, h, et in ((accA, hA, etA), (accB, hB, etB)):
                            for n in range(2):
                                nc.tensor.matmul(
                                    acc[:, n * 512 : (n + 1) * 512],
                                    vaug[kt_i][:, h, :],
                                    et[:, n * 512 : (n + 1) * 512],
                                    start=(kt_i == 0),
                                    stop=(kt_i == ST - 1),
                                )

                    def warmfill(ets):
                        # Zero-matmuls accumulating 0*et into acc: exact
                        # numeric no-ops that keep the PE array active in the
                        # exp-latency slivers, so the HAM activity monitor
                        # never re-throttles the PE clock to 1.2 GHz.
                        for i in range(fill):
                            nc.tensor.matmul(
                                accA[:, i * 512 : (i + 1) * 512],
                                zt[:],
                                ets[0][:, i * 512 : (i + 1) * 512],
                                start=False,
                                stop=False,
                                skip_group_check=True,
                            )

                    if jit_v is not None:
                        jit_v(0)
                    prev = lg(0)
                    for kt_i in range(1, ST):
                        if jit_v is not None:
                            jit_v(kt_i)
                        if extra is not None and kt_i in extra:
                            for f in extra[kt_i]:
                                f()
                        cur = lg(kt_i)
                        av(kt_i - 1, prev)
                        if fill and kt_i < ST - 1:
                            warmfill(prev)
                        prev = cur
                    av(ST - 1, prev)
                    # Stage acc out of PSUM fast (frees the banks for the
                    # next block so the PE never idles past the HAM window);
                    # the actual normalization happens in finish_norm(),
                    # emitted early in the NEXT block.
                    # Stage both accumulators out of PSUM in PARALLEL (head A
                    # via ScalarE, head B via DVE) so the acc slots free
                    # ~1.2us after the last exp and the next block's pipeline
                    # refills without a PE bubble (PE bubbles > ~1.5us here
                    # trip the HAM re-throttle and halve the PE clock).
                    soA = smp.tile([DEPTH + 1, QHS], f32, tag="so", name=f"soA{m}_{qh}")
                    soB = smp.tile([DEPTH + 1, QHS], f32, tag="so", name=f"soB{m}_{qh}")
                    nc.scalar.copy(soA[:], accA[:])
                    nc.vector.tensor_copy(soB[:], accB[:])

                    # Deferred normalization, split into small steps the
                    # caller schedules across the NEXT block's kt loop.
                    # reciprocal_approx_fast is broken on HW for partition-64
                    # sources, so the denominator row is first copied down to
                    # partition 0 with a stock DVE copy.
                    def steps(so, base, m=m, q0=q0):
                        sd = smp.tile([1, QHS], f32, tag="sd", name=f"sd{m}_{q0}_{base}")
                        r = smp.tile([1, QHS], f32, tag="r", name=f"r{m}_{q0}_{base}")
                        rb = smp.tile([64, QHS], f32, tag="rb", name=f"rb{m}_{q0}_{base}")

                        def s1():
                            nc.vector.tensor_copy(sd[:], so[DEPTH : DEPTH + 1, :])

                        def s2():
                            nc.vector.reciprocal_approx_fast(out=r[:], in_=sd[:])

                        def s3():
                            nc.gpsimd.partition_broadcast(rb[:], r[:])
                            mul_eng = nc.vector if tail_norm else nc.gpsimd
                            with nc.allow_low_precision(reason="fp16 pipeline"):
                                mul_eng.tensor_mul(
                                    outt[m][base : base + 64, q0 : q0 + QHS],
                                    so[0:DEPTH, :],
                                    rb[:],
                                )

                        return [s1, s2, s3]

                    return steps(soA, 0) + steps(soB, 64)

                # ---- emission ----
                wq_sb = load_w(wqt, "q")
                xq = load_x(qt, "q")
                wk_sb = load_w(wkt, "k")
                xk = load_x(kt, "k")
                wv_sb = load_w(wvt, "v")
                xv = load_x(vt, "v")
                nc.sync.dma_start(
                    out=wot_sb[:], in_=wot.rearrange("(m p) o -> p m o", p=128)
                )
                def wire(steps, *hooks, norm_kt=(3, 5, 7, 9, 11, 15)):
                    # steps: 6 deferred-normalize callables from the previous
                    # block; hooks: (kt, fn) pairs for this block.
                    ex = {}
                    if steps is not None:
                        for k, f in zip(norm_kt, steps):
                            ex.setdefault(k, []).append(f)
                    for k, f in hooks:
                        ex.setdefault(k, []).append(f)
                    return ex

                def projhooks(wq_s, xq_c, wk_s, xk_c, mi):
                    hooks = []
                    plan = [
                        (2, "q", 0, 0), (4, "q", 0, 1), (6, "q", 1, 0), (8, "q", 1, 1),
                        (10, "k", 0, 0), (12, "k", 0, 1), (13, "k", 1, 0), (14, "k", 1, 1),
                    ]
                    for kt, nm, sh, n in plan:
                        if nm == "q":
                            w_sb, xc, dst = wq_s, xq_c, qht
                        else:
                            w_sb, xc, dst = wk_s, xk_c, kht
                        hooks.append(
                            (kt, (lambda nm=nm, w_sb=w_sb, xc=xc, dst=dst, sh=sh, n=n:
                                  proj_qk_chunk(nm, w_sb, xc, dst, mi, sh, n)))
                        )
                    return hooks

                # Pre-warm the PE: a burst of zero-matmuls as soon as the
                # first weights land flips the HAM to 8/8 before the real
                # projections start, so they run at 2.4 GHz.
                pw = plog.tile([128, 384], f32, tag="plog", name="pw")
                for i in range(20):
                    nc.tensor.matmul(
                        pw[0:DEPTH + 1, :],
                        zt[:],
                        wq_sb[:, 0, :],
                        start=True,
                        stop=True,
                    )
                proj_qk_m("q", wq_sb, xq, qht, 0)
                proj_qk_m("k", wk_sb, xk, kht, 0)
                st = attnpair(0, 0, jit_v=lambda s: proj_v_tile(wv_sb, xv, s), fill=1)
                st = attnpair(0, 1, extra=wire(st, *projhooks(wq_sb, xq, wk_sb, xk, 1)))
                st = attnpair(1, 0, extra=wire(st, *projhooks(wq_sb, xq, wk_sb, xk, 2)))
                st = attnpair(1, 1, extra=wire(st), fill=2)
                st = attnpair(2, 0, extra=wire(st), fill=2, tail_norm=True)
                # outproj s=0..5 interleaved late into the last block (they
                # need block (2,0)'s normalize, which runs at kts 1..6 here
                # and completes by ~kt 9); s=6,7 right after the block while
                # its own normalize chain runs on DVE/GpSimd; s>=8 after.
                po = [
                    (10 + s, (lambda s=s: outproj(s, plog, on_act=True)))
                    for s in range(6)
                ]
                st = attnpair(
                    2, 1,
                    extra=wire(st, *po, norm_kt=(1, 2, 3, 4, 5, 6)),
                    fill=2,
                    tail_norm=True,
                )
                outproj(6, plog, on_act=True)
                outproj(7, plog, on_act=True)
                for f in st:
                    f()
                for s in range(8, ST):
                    outproj(s, plog, on_act=(s % 2 == 0))

    nc.compile()
    _CACHE["nc"] = nc
    return nc


def make_in_maps(v, k, q, wq, wk, wv, wo):
    f16 = lambda x: np.ascontiguousarray(x, dtype=np.float32).astype(np.float16)
    in_maps = []
    for c in range(8):
        b = c // 2
        hs = (c % 2) * HD
        in_maps.append(
            {
                "qt": f16(q[b].T),
                "kt": f16(k[b].T),
                "vt": f16(v[b].T),
                "wqt": f16(wq[hs : hs + HD, :].T),
                "wkt": f16(wk[hs : hs + HD, :].T),
                "wvt": f16(wv[hs : hs + HD, :].T),
                "wot": f16(wo[:, hs : hs + HD].T),
            }
        )
    return in_maps


def assemble(results, bo):
    y = np.empty((B, S, D), dtype=np.float32)
    for b in range(B):
        y[b] = results[2 * b]["y"] + results[2 * b + 1]["y"] + bo[None, :]
    return y


def kernel(v, k, q, wq, wk, wv, wo, bo):
    nc = _build()
    in_maps = make_in_maps(v, k, q, wq, wk, wv, wo)
    res = run_bass_kernel_spmd(nc, in_maps, list(range(8)))
    return assemble(res.results, np.asarray(bo, dtype=np.float32))


# revision 40
# speedup vs baseline: 1.1732x; 1.0097x over previous
"""Multi-head attention (B=4, S=2048, D=768, H=12) on 8 TRN2 NeuronCores.

Sharding: 48 (batch, head) units -> core c handles batch c//2, heads
6*(c%2) .. 6*(c%2)+5 (tensor-parallel over heads). Each core computes a
partial output projection; the host sums the two partials per batch and
adds the bias.

Design (vs the first working version):
- Attention processes HEAD PAIRS (even/odd head of each 128-dim m tile)
  with q split in halves: per (m, qh) block, per kt tile: two K=64 logit
  matmuls (disjoint PE row groups), one exp per head, four attn@V
  matmuls accumulating into per-head [65, 1024] PSUM accumulators whose
  65th row (ones appended to V) yields the softmax denominator.
- exp runs on TWO engines in parallel: head A on ScalarE (table exp),
  head B on the Vector engine via the fp16-bitcast trick:
  i16 = (int16)(1024*(logit*scale*log2e + 15) - 44), bitcast to fp16
  ~= 2^t (max rel err ~3%; the softmax ratio cancels the bias). One
  fused DVE tensor_scalar per tile -> a second exp pipe.
- The PE's HAM activity monitor halves the PE clock whenever PE duty
  dips in a ~3.4us window, and a dependency-stalled kernel then locks
  in cold. Countermeasures: a pre-warm burst of zero-matmuls at
  startup, zero-matmul warm filler (0*et accumulated into the live
  accumulators) in blocks with no projection work, fast parallel
  staging of the accumulators out of PSUM at block ends (ScalarE +
  DVE), and deferred normalization spread across the next block.
- reciprocal via reciprocal_approx_fast from a partition-0 staged copy
  (the custom DVE op mis-reads partition-64 sources on HW).
- Q/K projections for m=1,2, and the first output projections, are
  interleaved INTO attention blocks as [128,512] PSUM chunks that fill
  the PE gaps the exp latency leaves; remaining outprojs drain at the
  tail with copies alternating between ScalarE and DVE.
- Outputs ship as fp16 (halves the output DMA); the host sums the two
  per-batch partials in fp32 and adds the bias.
"""

import numpy as np

import concourse.bacc as bacc
import concourse.mybir as mybir
from concourse import tile
from concourse.bass_utils import run_bass_kernel_spmd

B, S, D, H = 4, 2048, 768, 12
DEPTH = D // H  # 64
HPC = H // 2  # heads per core: 6
HD = HPC * DEPTH  # per-core projected dim: 384
EC = D // 128  # e chunks: 6
MT = HD // 128  # d tiles: 3
ST = S // 128  # s tiles: 16
QH = 2  # q halves in attention
QHS = S // QH  # 1024

f32 = mybir.dt.float32
fp16 = mybir.dt.float16
i16 = mybir.dt.int16
AF = mybir.ActivationFunctionType
ALU = mybir.AluOpType

SCALE = 1.0 / np.sqrt(DEPTH)
# DVE fast-exp constants: exp(x) ~ bitcast_fp16(int16(1024*(x*SCALE*log2e + 15) - 44))
EXP_A = float(SCALE * np.log2(np.e) * 1024.0)
EXP_B = float(15.0 * 1024.0 - 44.0)
_CACHE = {}


def _build():
    if "nc" in _CACHE:
        return _CACHE["nc"]
    nc = bacc.Bacc("TRN2", target_bir_lowering=False, debug=False, num_devices=8)
    qt = nc.dram_tensor("qt", [D, S], fp16, kind="ExternalInput").ap()
    kt = nc.dram_tensor("kt", [D, S], fp16, kind="ExternalInput").ap()
    vt = nc.dram_tensor("vt", [D, S], fp16, kind="ExternalInput").ap()
    wqt = nc.dram_tensor("wqt", [D, HD], fp16, kind="ExternalInput").ap()
    wkt = nc.dram_tensor("wkt", [D, HD], fp16, kind="ExternalInput").ap()
    wvt = nc.dram_tensor("wvt", [D, HD], fp16, kind="ExternalInput").ap()
    wot = nc.dram_tensor("wot", [HD, D], fp16, kind="ExternalInput").ap()
    y = nc.dram_tensor("y", [S, D], fp16, kind="ExternalOutput").ap()

    with tile.TileContext(nc) as tc:
        with (
            tc.tile_pool(name="wp", bufs=3) as wp,
            tc.tile_pool(name="wop", bufs=1) as wop,
            tc.tile_pool(name="xp", bufs=3 * EC) as xp,
            tc.tile_pool(name="qk", bufs=2 * MT) as qkp,
            tc.tile_pool(name="vg", bufs=ST) as vgp,
            tc.tile_pool(name="ot", bufs=MT) as otp,
            tc.tile_pool(name="ep", bufs=8) as epp,
            tc.tile_pool(name="sm", bufs=2) as smp,
            tc.tile_pool(name="yp", bufs=2) as ypp,
        ):
            # ---- persistent SBUF tensors ----
            qht = [qkp.tile([128, S], fp16, tag="qk", name=f"qht{i}") for i in range(MT)]
            kht = [qkp.tile([128, S], fp16, tag="qk", name=f"kht{i}") for i in range(MT)]
            vaug = [vgp.tile([128, HPC, DEPTH + 1], fp16, tag="vg", name=f"vaug{i}") for i in range(ST)]
            outt = [otp.tile([128, S], fp16, tag="ot", name=f"outt{i}") for i in range(MT)]

            wot_sb = wop.tile([128, MT, D], fp16, tag="wot")
            zt = wop.tile([128, DEPTH + 1], fp16, tag="zt")
            nc.vector.memset(zt[:], 0.0)

            def load_w(wdram, nm):
                w_sb = wp.tile([128, EC, HD], fp16, tag="w", name=f"w_{nm}")
                for ci in range(EC):
                    nc.sync.dma_start(
                        out=w_sb[:, ci, :],
                        in_=wdram[ci * 128 : (ci + 1) * 128, :],
                    )
                return w_sb

            def load_x(xdram, nm):
                xc = [
                    xp.tile([128, S], fp16, tag="x", name=f"x{nm}_{i}")
                    for i in range(EC)
                ]
                for ci in range(EC):
                    nc.sync.dma_start(
                        out=xc[ci][:], in_=xdram[ci * 128 : (ci + 1) * 128, :]
                    )
                return xc

            with (
                tc.tile_pool(name="plog", bufs=2, space="PSUM") as plog,
                tc.tile_pool(name="pacc", bufs=2, space="PSUM") as pacc,
            ):

                def proj_qk_m(name, w_sb, xc, dst, m):
                    # full Q/K d-tile (m): both 1024-halves (startup only)
                    for sh in range(2):
                        proj_qk_chunk(name, w_sb, xc, dst, m, sh, 0)
                        proj_qk_chunk(name, w_sb, xc, dst, m, sh, 1)

                def proj_qk_chunk(name, w_sb, xc, dst, m, sh, n):
                    # one [128, 512] chunk of a Q/K projection d-tile
                    off = sh * 1024 + n * 512
                    pt = plog.tile(
                        [128, 512], f32, tag="plog", name=f"p{name}{m}_{sh}_{n}"
                    )
                    for ci in range(EC):
                        nc.tensor.matmul(
                            pt[:],
                            w_sb[:, ci, m * 128 : (m + 1) * 128],
                            xc[ci][:, off : off + 512],
                            start=(ci == 0),
                            stop=(ci == EC - 1),
                        )
                    with nc.allow_low_precision(reason="fp16 pipeline"):
                        nc.scalar.copy(dst[m][:, off : off + 512], pt[:])

                def proj_v_tile(wv_sb, xc, s):
                    pv = plog.tile([128, HD], f32, tag="plog", name=f"pv{s}")
                    for ci in range(EC):
                        nc.tensor.matmul(
                            pv[:],
                            xc[ci][:, s * 128 : (s + 1) * 128],
                            wv_sb[:, ci, :],
                            start=(ci == 0),
                            stop=(ci == EC - 1),
                        )
                    with nc.allow_low_precision(reason="fp16 pipeline"):
                        nc.vector.tensor_copy(
                            vaug[s][:, :, 0:DEPTH],
                            pv[:].rearrange("p (h d) -> p h d", d=DEPTH),
                        )
                    nc.vector.memset(vaug[s][:, :, DEPTH : DEPTH + 1], 1.0)

                def outproj(s, pool, on_act=False):
                    py = pool.tile([128, D], f32, tag=pool.name, name=f"py{s}")
                    for m in range(MT):
                        for n0, n1 in ((0, 512), (512, 768)):
                            nc.tensor.matmul(
                                py[:, n0:n1],
                                outt[m][:, s * 128 : (s + 1) * 128],
                                wot_sb[:, m, n0:n1],
                                start=(m == 0),
                                stop=(m == MT - 1),
                            )
                    ty = ypp.tile([128, D], fp16, tag="y", name=f"ty{s}")
                    with nc.allow_low_precision(reason="fp16 output"):
                        if on_act:
                            nc.scalar.copy(ty[:], py[:])
                        else:
                            nc.vector.tensor_copy(ty[:], py[:])
                    nc.sync.dma_start(out=y[s * 128 : (s + 1) * 128, :], in_=ty[:])

                def attnpair(m, qh, jit_v=None, extra=None, fill=0, tail_norm=False):
                    # heads A=2m (rows 0:64), B=2m+1 (rows 64:128)
                    hA, hB = 2 * m, 2 * m + 1
                    q0 = qh * QHS
                    accA = pacc.tile(
                        [DEPTH + 1, QHS], f32, tag="pacc", name=f"accA{m}_{qh}"
                    )
                    accB = pacc.tile(
                        [DEPTH + 1, QHS], f32, tag="pacc", name=f"accB{m}_{qh}"
                    )

                    def lg(kt_i):
                        lpA = plog.tile(
                            [128, QHS], f32, tag="plog", name=f"lpA{m}_{qh}_{kt_i}"
                        )
                        lpB = plog.tile(
                            [128, QHS], f32, tag="plog", name=f"lpB{m}_{qh}_{kt_i}"
                        )
                        for n in range(2):
                            for base, lp in ((0, lpA), (64, lpB)):
                                nc.tensor.matmul(
                                    lp[:, n * 512 : (n + 1) * 512],
                                    kht[m][
                                        base : base + 64,
                                        kt_i * 128 : (kt_i + 1) * 128,
                                    ],
                                    qht[m][
                                        base : base + 64,
                                        q0 + n * 512 : q0 + (n + 1) * 512,
                                    ],
                                    start=True,
                                    stop=True,
                                )
                        etA = epp.tile(
                            [128, QHS], fp16, tag="ep", name=f"etA{m}_{qh}_{kt_i}"
                        )
                        etB = epp.tile(
                            [128, QHS], fp16, tag="ep", name=f"etB{m}_{qh}_{kt_i}"
                        )
                        with nc.allow_low_precision(reason="fp16 pipeline"):
                            nc.scalar.activation(etA[:], lpA[:], AF.Exp, scale=SCALE)
                        # head B's exp always on the DVE (2^t bitcast trick):
                        # a second exp pipe in parallel with ScalarE.
                        nc.vector.tensor_scalar(
                            out=etB[:].bitcast(i16),
                            in0=lpB[:],
                            scalar1=EXP_A,
                            scalar2=EXP_B,
                            op0=ALU.mult,
                            op1=ALU.add,
                        )
                        return etA, etB

                    def av(kt_i, ets):
                        etA, etB = ets
                        for acc, h, et in ((accA, hA, etA), (accB, hB, etB)):
                            for n in range(2):
                                nc.tensor.matmul(
                                    acc[:, n * 512 : (n + 1) * 512],
                                    vaug[kt_i][:, h, :],
                                    et[:, n * 512 : (n + 1) * 512],
                                    start=(kt_i == 0),
                                    stop=(kt_i == ST - 1),
                                )

                    def warmfill(ets):
                        # Zero-matmuls accumulating 0*et into acc: exact
                        # numeric no-ops that keep the PE array active in the
                        # exp-latency slivers, so the HAM activity monitor
                        # never re-throttles the PE clock to 1.2 GHz.
                        for i in range(fill):
                            nc.tensor.matmul(
                                accA[:, i * 512 : (i + 1) * 512],
                                zt[:],
                                ets[0][:, i * 512 : (i + 1) * 512],
                                start=False,
                                stop=False,
                                skip_group_check=True,
                            )

                    if jit_v is not None:
                        jit_v(0)
                    prev = lg(0)
                    for kt_i in range(1, ST):
                        if jit_v is not None:
                            jit_v(kt_i)
                        if extra is not None and kt_i in extra:
                            for f in extra[kt_i]:
                                f()
                        cur = lg(kt_i)
                        av(kt_i - 1, prev)
                        if fill and kt_i < ST - 1:
                            warmfill(prev)
                        prev = cur
                    av(ST - 1, prev)
                    # Stage acc out of PSUM fast (frees the banks for the
                    # next block so the PE never idles past the HAM window);
                    # the actual normalization happens in finish_norm(),
                    # emitted early in the NEXT block.
                    # Stage both accumulators out of PSUM in PARALLEL (head A
                    # via ScalarE, head B via DVE) so the acc slots free
                    # ~1.2us after the last exp and the next block's pipeline
                    # refills without a PE bubble (PE bubbles > ~1.5us here
                    # trip the HAM re-throttle and halve the PE clock).
                    soA = smp.tile([DEPTH + 1, QHS], f32, tag="so", name=f"soA{m}_{qh}")
                    soB = smp.tile([DEPTH + 1, QHS], f32, tag="so", name=f"soB{m}_{qh}")
                    nc.scalar.copy(soA[:], accA[:])
                    nc.vector.tensor_copy(soB[:], accB[:])

                    # Deferred normalization, split into small steps the
                    # caller schedules across the NEXT block's kt loop.
                    # reciprocal_approx_fast is broken on HW for partition-64
                    # sources, so the denominator row is first copied down to
                    # partition 0 with a stock DVE copy.
                    def steps(so, base, m=m, q0=q0):
                        sd = smp.tile([1, QHS], f32, tag="sd", name=f"sd{m}_{q0}_{base}")
                        r = smp.tile([1, QHS], f32, tag="r", name=f"r{m}_{q0}_{base}")
                        rb = smp.tile([64, QHS], f32, tag="rb", name=f"rb{m}_{q0}_{base}")

                        def s1():
                            nc.vector.tensor_copy(sd[:], so[DEPTH : DEPTH + 1, :])

                        def s2():
                            nc.vector.reciprocal_approx_fast(out=r[:], in_=sd[:])

                        def s3():
                            nc.gpsimd.partition_broadcast(rb[:], r[:])
                            mul_eng = nc.vector if tail_norm else nc.gpsimd
                            with nc.allow_low_precision(reason="fp16 pipeline"):
                                mul_eng.tensor_mul(
                                    outt[m][base : base + 64, q0 : q0 + QHS],
                                    so[0:DEPTH, :],
                                    rb[:],
                                )

                        return [s1, s2, s3]

                    return steps(soA, 0) + steps(soB, 64)

                # ---- emission ----
                wq_sb = load_w(wqt, "q")
                xq = load_x(qt, "q")
                wk_sb = load_w(wkt, "k")
                xk = load_x(kt, "k")
                wv_sb = load_w(wvt, "v")
                xv = load_x(vt, "v")
                nc.sync.dma_start(
                    out=wot_sb[:], in_=wot.rearrange("(m p) o -> p m o", p=128)
                )
                def wire(steps, *hooks, norm_kt=(3, 5, 7, 9, 11, 15)):
                    # steps: 6 deferred-normalize callables from the previous
                    # block; hooks: (kt, fn) pairs for this block.
                    ex = {}
                    if steps is not None:
                        for k, f in zip(norm_kt, steps):
                            ex.setdefault(k, []).append(f)
                    for k, f in hooks:
                        ex.setdefault(k, []).append(f)
                    return ex

                def projhooks(wq_s, xq_c, wk_s, xk_c, mi):
                    hooks = []
                    plan = [
                        (2, "q", 0, 0), (4, "q", 0, 1), (6, "q", 1, 0), (8, "q", 1, 1),
                        (10, "k", 0, 0), (12, "k", 0, 1), (13, "k", 1, 0), (14, "k", 1, 1),
                    ]
                    for kt, nm, sh, n in plan:
                        if nm == "q":
                            w_sb, xc, dst = wq_s, xq_c, qht
                        else:
                            w_sb, xc, dst = wk_s, xk_c, kht
                        hooks.append(
                            (kt, (lambda nm=nm, w_sb=w_sb, xc=xc, dst=dst, sh=sh, n=n:
                                  proj_qk_chunk(nm, w_sb, xc, dst, mi, sh, n)))
                        )
                    return hooks

                # Pre-warm the PE: a burst of zero-matmuls as soon as the
                # first weights land flips the HAM to 8/8 before the real
                # projections start, so they run at 2.4 GHz.
                pw = plog.tile([128, 384], f32, tag="plog", name="pw")
                for i in range(20):
                    nc.tensor.matmul(
                        pw[0:DEPTH + 1, :],
                        zt[:],
                        wq_sb[:, 0, :],
                        start=True,
                        stop=True,
                    )
                proj_qk_m("q", wq_sb, xq, qht, 0)
                proj_qk_m("k", wk_sb, xk, kht, 0)
                st = attnpair(0, 0, jit_v=lambda s: proj_v_tile(wv_sb, xv, s), fill=1)
                st = attnpair(0, 1, extra=wire(st, *projhooks(wq_sb, xq, wk_sb, xk, 1)))
                st = attnpair(1, 0, extra=wire(st, *projhooks(wq_sb, xq, wk_sb, xk, 2)))
                st = attnpair(1, 1, extra=wire(st), fill=2)
                st = attnpair(2, 0, extra=wire(st), fill=2, tail_norm=True)
                # outproj s=0..5 interleaved late into the last block (they
                # need block (2,0)'s normalize, which runs at kts 1..6 here
                # and completes by ~kt 9); s=6,7 right after the block while
                # its own normalize chain runs on DVE/GpSimd; s>=8 after.
                po = [
                    (10 + s, (lambda s=s: outproj(s, plog, on_act=True)))
                    for s in range(6)
                ]
                st = attnpair(
                    2, 1,
                    extra=wire(st, *po, norm_kt=(1, 2, 3, 4, 5, 6)),
                    fill=2,
                    tail_norm=True,
                )
                outproj(6, plog, on_act=True)
                outproj(7, plog, on_act=True)
                for f in st:
                    f()
                for s in range(8, ST):
                    outproj(s, plog, on_act=(s % 2 == 0))

    nc.compile()
    _CACHE["nc"] = nc
    return nc


def make_in_maps(v, k, q, wq, wk, wv, wo):
    f16 = lambda x: np.ascontiguousarray(x, dtype=np.float32).astype(np.float16)
    in_maps = []
    for c in range(8):
        b = c // 2
        hs = (c % 2) * HD
        in_maps.append(
            {
                "qt": f16(q[b].T),
                "kt": f16(k[b].T),
                "vt": f16(v[b].T),
                "wqt": f16(wq[hs : hs + HD, :].T),
                "wkt": f16(wk[hs : hs + HD, :].T),
                "wvt": f16(wv[hs : hs + HD, :].T),
                "wot": f16(wo[:, hs : hs + HD].T),
            }
        )
    return in_maps


def assemble(results, bo):
    y = np.empty((B, S, D), dtype=np.float32)
    for b in range(B):
        y[b] = (
            results[2 * b]["y"].astype(np.float32)
            + results[2 * b + 1]["y"].astype(np.float32)
            + bo[None, :]
        )
    return y


def kernel(v, k, q, wq, wk, wv, wo, bo):
    nc = _build()
    in_maps = make_in_maps(v, k, q, wq, wk, wv, wo)
    res = run_bass_kernel_spmd(nc, in_maps, list(range(8)))
    return assemble(res.results, np.asarray(bo, dtype=np.float32))
